# revision 2
# baseline (speedup 1.0000x reference)
"""BERT self-attention (B=4, S=2048, E=768, H=12) on 8 TRN2 NeuronCores.

Sharding: (batch, head-half) — core c handles batch c//2, heads 6*(c%2)..+6.
Each core is fully independent (no collectives).

Device-side structure (per core):
  - inputs arrive pre-transposed from host: xT [768,2048], W*T [768,384]
    (Wq/bq pre-scaled by 1/8 = 1/sqrt(D)), dmaskT [2048,2048].
  - projections (bf16): qT,kT [o,m] layout; v [m,o] layout augmented with a
    ones column per head (gives softmax denominators through the PV matmul).
  - scores^T[k,q] = kT.T @ qT per (head-pair, k-chunk, q-chunk), two heads
    row-packed in one PE pass (contraction d=64 each).
  - one ACT pass: exp(scores) PSUM -> SBUF bf16 (the kernel bottleneck).
  - masks enter via E_T = exp(dmaskT + amask_k) (one-time ACT pass),
    multiplied in at bf16 on DVE: prod = exp_s * E_T.
  - PV: ctx_u^T[65,q] = v_aug.T @ prod accumulated over k-chunks; row 64 is
    the softmax denominator.
  - PE-transpose ctx_u^T -> [q,65], divide rows 0..63 by row 64 (DVE
    reciprocal + tensor_scalar), assemble [q, 384] f32, DMA out.
"""

import sys

if "/opt/trn_rl_repo" not in sys.path:
    sys.path.insert(0, "/opt/trn_rl_repo")

from contextlib import ExitStack

import numpy as np

import concourse.bass as bass
import concourse.tile as tile
from concourse import bacc, mybir
from concourse.bass_utils import run_bass_kernel_spmd
from concourse.masks import make_identity

B, S, E, H = 4, 2048, 768, 12
D = 64
N_CORES = 8
HPC = 6            # heads per core
EC = HPC * D       # 384 embedding cols per core
NIC = E // 128     # 6 contraction chunks
NOC = EC // 128    # 3 output chunks (= head pairs)
NKC = S // 128     # 16 k chunks
QW = 512           # q tile width
NQQ = S // QW      # 4 q chunks

F32 = mybir.dt.float32
BF16 = mybir.dt.bfloat16
Exp = mybir.ActivationFunctionType.Exp


def _emit(ctx: ExitStack, tc: tile.TileContext, h):
    nc = tc.nc

    persist = ctx.enter_context(tc.tile_pool(name="persist", bufs=1))
    consts = ctx.enter_context(tc.tile_pool(name="consts", bufs=1))

    # ---- constants ----
    idt = consts.tile([128, 128], BF16)
    make_identity(nc, idt[:])
    amask_sb = consts.tile([128, NKC], F32)
    nc.sync.dma_start(out=amask_sb[:], in_=h["amask"].ap())
    bq_sb = consts.tile([128, NOC], F32)
    nc.sync.dma_start(out=bq_sb[:], in_=h["bq"].ap())
    bk_sb = consts.tile([128, NOC], F32)
    nc.sync.dma_start(out=bk_sb[:], in_=h["bk"].ap())
    bv_f32 = consts.tile([1, EC], F32)
    nc.sync.dma_start(out=bv_f32[:], in_=h["bv"].ap())
    bv_sb = consts.tile([1, EC], BF16)
    nc.vector.tensor_copy(bv_sb[:], bv_f32[:])
    ones1 = consts.tile([1, 128], BF16)
    nc.vector.memset(ones1[:], 1.0)

    # ---- persistent activations ----
    qT = persist.tile([128, NOC, S], BF16)        # [o%128, o-chunk, m]
    kT = persist.tile([128, NOC, S], BF16)
    vaug = persist.tile([128, NKC, HPC, D + 1], BF16)  # [m%128, m-chunk, head, d|one]
    ET = persist.tile([128, NKC, S], BF16)        # exp(dmaskT + amask), [k%128, k-chunk, q]

    nc.vector.memset(vaug[:, :, :, D : D + 1], 1.0)

    # ---- stage A: load + bf16-convert xT and W*T ----
    with tc.tile_pool(name="stageAB", bufs=1) as sab, \
         tc.tile_pool(name="stg", bufs=3) as stg, \
         tc.tile_pool(name="proj_psum", bufs=2, space="PSUM") as pps:
        xTb = sab.tile([128, NIC, S], BF16)
        wqb = sab.tile([128, NIC, EC], BF16)
        wkb = sab.tile([128, NIC, EC], BF16)
        wvb = sab.tile([128, NIC, EC], BF16)
        for ic in range(NIC):
            xs = stg.tile([128, S], F32)
            nc.sync.dma_start(out=xs[:], in_=h["xT"].ap()[ic * 128 : (ic + 1) * 128, :])
            nc.vector.tensor_copy(xTb[:, ic, :], xs[:])
        for name, wtb in (("wqT", wqb), ("wkT", wkb), ("wvT", wvb)):
            for ic in range(NIC):
                ws = stg.tile([128, EC], F32)
                nc.sync.dma_start(
                    out=ws[:], in_=h[name].ap()[ic * 128 : (ic + 1) * 128, :]
                )
                nc.vector.tensor_copy(wtb[:, ic, :], ws[:])

        # ---- stage B: projections ----
        for dst, wtb, bias in ((qT, wqb, bq_sb), (kT, wkb, bk_sb)):
            for oc in range(NOC):
                for mq in range(NQQ):
                    ps = pps.tile([128, QW], F32)
                    for ic in range(NIC):
                        nc.tensor.matmul(
                            ps[:],
                            wtb[:, ic, oc * 128 : (oc + 1) * 128],
                            xTb[:, ic, mq * QW : (mq + 1) * QW],
                            start=(ic == 0),
                            stop=(ic == NIC - 1),
                        )
                    nc.vector.tensor_scalar_add(
                        dst[:, oc, mq * QW : (mq + 1) * QW], ps[:], bias[:, oc : oc + 1]
                    )
        for mc in range(NKC):
            vps = pps.tile([128, EC], F32)
            for ic in range(NIC):
                nc.tensor.matmul(
                    vps[:],
                    xTb[:, ic, mc * 128 : (mc + 1) * 128],
                    wvb[:, ic, :],
                    start=(ic == 0),
                    stop=False,
                )
            nc.tensor.matmul(vps[:], ones1[:], bv_sb[:], start=False, stop=True)
            nc.vector.tensor_copy(
                vaug[:, mc, :, 0:D], vps[:].rearrange("p (h d) -> p h d", h=HPC)
            )

    # ---- stage C: E_T = exp(dmaskT + amask_k) ----
    with tc.tile_pool(name="cstg", bufs=2) as cstg:
        for kc in range(NKC):
            dm = cstg.tile([128, S], F32)
            nc.sync.dma_start(
                out=dm[:], in_=h["dmaskT"].ap()[kc * 128 : (kc + 1) * 128, :]
            )
            nc.scalar.activation(ET[:, kc, :], dm[:], Exp, bias=amask_sb[:, kc : kc + 1])

    # ---- stage D: attention ----
    with tc.tile_pool(name="s_psum", bufs=2, space="PSUM") as sps, \
         tc.tile_pool(name="ctx_psum", bufs=1, space="PSUM") as cps, \
         tc.tile_pool(name="tp_psum", bufs=2, space="PSUM") as tps, \
         tc.tile_pool(name="dwork", bufs=3) as dwork, \
         tc.tile_pool(name="owork", bufs=8) as owork:
        for qq in range(NQQ):
            qs = slice(qq * QW, (qq + 1) * QW)
            osb = []
            for t in range(4):
                ot = owork.tile([128, EC], F32, tag="osb")
                osb.append(ot)
            for j in range(NOC):
                ctxA = cps.tile([D + 1, QW], F32, tag="ctxA")
                ctxB = cps.tile([D + 1, QW], F32, tag="ctxB")
                for kc in range(NKC):
                    ks = slice(kc * 128, (kc + 1) * 128)
                    sab_t = sps.tile([128, 2 * QW], F32, tag="S")
                    nc.tensor.matmul(
                        sab_t[:, 0:QW],
                        kT[0:64, j, ks],
                        qT[0:64, j, qs],
                        start=True,
                        stop=True,
                        tile_position=(0, 0),
                    )
                    nc.tensor.matmul(
                        sab_t[:, QW : 2 * QW],
                        kT[64:128, j, ks],
                        qT[64:128, j, qs],
                        start=True,
                        stop=True,
                        tile_position=(64, 0),
                    )
                    ex = dwork.tile([128, 2 * QW], BF16, tag="ex")
                    nc.scalar.activation(ex[:], sab_t[:], Exp)
                    pr = dwork.tile([128, 2 * QW], BF16, tag="pr")
                    nc.vector.tensor_mul(pr[:, 0:QW], ex[:, 0:QW], ET[:, kc, qs])
                    nc.vector.tensor_mul(
                        pr[:, QW : 2 * QW], ex[:, QW : 2 * QW], ET[:, kc, qs]
                    )
                    nc.tensor.matmul(
                        ctxA[:],
                        vaug[:, kc, 2 * j, :],
                        pr[:, 0:QW],
                        start=(kc == 0),
                        stop=(kc == NKC - 1),
                    )
                    nc.tensor.matmul(
                        ctxB[:],
                        vaug[:, kc, 2 * j + 1, :],
                        pr[:, QW : 2 * QW],
                        start=(kc == 0),
                        stop=(kc == NKC - 1),
                    )
                for hh, cpsum in ((0, ctxA), (1, ctxB)):
                    csb = dwork.tile([D + 1, QW], BF16, tag="csb")
                    nc.vector.tensor_copy(csb[:], cpsum[:])
                    col = (2 * j + hh) * D
                    for t in range(4):
                        tp = tps.tile([128, D + 1], BF16, tag="tp")
                        nc.tensor.transpose(
                            tp[:], csb[:, t * 128 : (t + 1) * 128], idt[0 : D + 1, 0 : D + 1]
                        )
                        rc = dwork.tile([128, 1], F32, tag="rc")
                        nc.vector.reciprocal(rc[:], tp[:, D : D + 1])
                        nc.vector.tensor_scalar_mul(
                            osb[t][:, col : col + D], tp[:, 0:D], rc[:]
                        )
            for t in range(4):
                q0 = qq * QW + t * 128
                nc.sync.dma_start(out=h["out"].ap()[q0 : q0 + 128, :], in_=osb[t][:])


def build():
    nc = bacc.Bacc("TRN2", target_bir_lowering=False, debug=False, num_devices=N_CORES)
    h = {
        "xT": nc.dram_tensor("xT", [E, S], F32, kind="ExternalInput"),
        "wqT": nc.dram_tensor("wqT", [E, EC], F32, kind="ExternalInput"),
        "wkT": nc.dram_tensor("wkT", [E, EC], F32, kind="ExternalInput"),
        "wvT": nc.dram_tensor("wvT", [E, EC], F32, kind="ExternalInput"),
        "bq": nc.dram_tensor("bq", [128, NOC], F32, kind="ExternalInput"),
        "bk": nc.dram_tensor("bk", [128, NOC], F32, kind="ExternalInput"),
        "bv": nc.dram_tensor("bv", [1, EC], F32, kind="ExternalInput"),
        "amask": nc.dram_tensor("amask", [128, NKC], F32, kind="ExternalInput"),
        "dmaskT": nc.dram_tensor("dmaskT", [S, S], F32, kind="ExternalInput"),
        "out": nc.dram_tensor("out", [S, EC], F32, kind="ExternalOutput"),
    }
    with tile.TileContext(nc) as tc:
        with ExitStack() as ctx:
            _emit(ctx, tc, h)
    nc.compile()
    return nc


def prep_in_maps(inputs):
    hs = np.asarray(inputs["hidden_states"], dtype=np.float32)
    am = np.asarray(inputs["attention_mask"], dtype=np.float32)
    dm = np.asarray(inputs["domain_attn_mask"], dtype=np.float32)
    Wq = np.asarray(inputs["Wq"], dtype=np.float32)
    bq = np.asarray(inputs["bq"], dtype=np.float32)
    Wk = np.asarray(inputs["Wk"], dtype=np.float32)
    bk = np.asarray(inputs["bk"], dtype=np.float32)
    Wv = np.asarray(inputs["Wv"], dtype=np.float32)
    bv = np.asarray(inputs["bv"], dtype=np.float32)

    in_maps = []
    for c in range(N_CORES):
        b = c // 2
        e0 = (c % 2) * EC
        sl = slice(e0, e0 + EC)
        in_maps.append(
            {
                "xT": np.ascontiguousarray(hs[b].T),
                "wqT": np.ascontiguousarray(Wq[sl, :].T) * 0.125,
                "wkT": np.ascontiguousarray(Wk[sl, :].T),
                "wvT": np.ascontiguousarray(Wv[sl, :].T),
                "bq": np.ascontiguousarray((bq[sl] * 0.125).reshape(NOC, 128).T),
                "bk": np.ascontiguousarray(bk[sl].reshape(NOC, 128).T),
                "bv": bv[sl].reshape(1, EC).copy(),
                "amask": np.ascontiguousarray(am[b, 0, 0, :].reshape(NKC, 128).T),
                "dmaskT": np.ascontiguousarray(dm[b, 0].T),
            }
        )
    return in_maps


_cached_nc = None


def run(inputs, trace=False):
    global _cached_nc
    if _cached_nc is None:
        _cached_nc = build()
    in_maps = prep_in_maps(inputs)
    res = run_bass_kernel_spmd(
        _cached_nc, in_maps, core_ids=list(range(N_CORES)), trace=trace
    )
    out = np.empty((B, S, E), dtype=np.float32)
    for c in range(N_CORES):
        b = c // 2
        e0 = (c % 2) * EC
        out[b, :, e0 : e0 + EC] = res.results[c]["out"]
    return out, res


def kernel(**inputs) -> np.ndarray:
    return run(inputs)[0]


# revision 4
# speedup vs baseline: 1.1737x; 1.1737x over previous
"""BERT self-attention (B=4, S=2048, E=768, H=12) on 8 TRN2 NeuronCores.

Sharding: (batch, head-half) — core c handles batch c//2, heads 6*(c%2)..+6.
Each core is fully independent (no collectives).

Device-side structure (per core):
  - inputs arrive pre-transposed from host: xT [768,2048], W*T [768,384]
    (Wq/bq pre-scaled by 1/8 = 1/sqrt(D)), dmaskT [2048,2048].
  - x/W loaded via SWDGE cast-DMA (f32 HBM -> bf16 SBUF) on the gpsimd
    queue; dmaskT streamed f32 on the sync queue (separate DMA queues).
  - projections (bf16): qT,kT in [o,m] layout; v in [m,o] layout augmented
    with a ones column per head (softmax denominators via the PV matmul).
  - scores^T[k,q] = kT.T @ qT, two heads row-packed per PE pass (d=64 each).
  - one ACT pass per k-chunk: exp(scores) PSUM -> SBUF bf16 (bottleneck).
  - masks enter via E_T = exp(dmaskT + amask_k) (ACT, overlapped with the
    projections), multiplied in at bf16: head A on DVE, head B on GpSimd.
  - PV: ctx_u^T[65,q] = v_aug.T @ prod accumulated over k-chunks; row 64 is
    the softmax denominator.
  - PE-transpose ctx_u^T -> [q,65] (batched per 4 q-subtiles), divide rows
    0..63 by row 64 (DVE reciprocal + one broadcast multiply), one DMA per
    512-row output stripe.
"""

import sys

if "/opt/trn_rl_repo" not in sys.path:
    sys.path.insert(0, "/opt/trn_rl_repo")

from contextlib import ExitStack

import numpy as np

import concourse.bass as bass
import concourse.tile as tile
from concourse import bacc, mybir
from concourse.bass_utils import run_bass_kernel_spmd
from concourse.masks import make_identity

B, S, E, H = 4, 2048, 768, 12
D = 64
N_CORES = 8
HPC = 6            # heads per core
EC = HPC * D       # 384 embedding cols per core
NIC = E // 128     # 6 contraction chunks
NOC = EC // 128    # 3 output chunks (= head pairs)
NKC = S // 128     # 16 k chunks
QW = 512           # q tile width
NQQ = S // QW      # 4 q chunks

F32 = mybir.dt.float32
BF16 = mybir.dt.bfloat16
Exp = mybir.ActivationFunctionType.Exp


def _bcast_last(ap: bass.AP, n: int) -> bass.AP:
    """Append a step-0 broadcast dim of size n to an AP."""
    return bass.AP(tensor=ap.tensor, offset=ap.offset, ap=[*ap.ap, [0, n]])


def _emit(ctx: ExitStack, tc: tile.TileContext, h):
    nc = tc.nc

    persist = ctx.enter_context(tc.tile_pool(name="persist", bufs=1))
    consts = ctx.enter_context(tc.tile_pool(name="consts", bufs=1))

    # ---- constants ----
    idt = consts.tile([128, 128], BF16)
    make_identity(nc, idt[:])
    amask_sb = consts.tile([128, NKC], F32)
    nc.sync.dma_start(out=amask_sb[:], in_=h["amask"].ap())
    bq_sb = consts.tile([128, NOC], F32)
    nc.sync.dma_start(out=bq_sb[:], in_=h["bq"].ap())
    bk_sb = consts.tile([128, NOC], F32)
    nc.sync.dma_start(out=bk_sb[:], in_=h["bk"].ap())
    bv_sb = consts.tile([1, EC], BF16)
    nc.gpsimd.dma_start(out=bv_sb[:], in_=h["bv"].ap())  # cast f32->bf16
    ones1 = consts.tile([1, 128], BF16)
    nc.vector.memset(ones1[:], 1.0)

    # ---- persistent activations ----
    qT = persist.tile([128, NOC, S], BF16)        # [o%128, o-chunk, m]
    kT = persist.tile([128, NOC, S], BF16)
    vaug = persist.tile([128, NKC, HPC, D + 4], BF16)  # [m%128, m-chunk, head, d|one]
    ET = persist.tile([128, NKC, S], BF16)        # exp(dmaskT + amask), [k%128, k-chunk, q]

    nc.vector.memset(vaug[:, :, :, D : D + 1], 1.0)

    cstg = ctx.enter_context(tc.tile_pool(name="cstg", bufs=2))

    # ---- stage C: E_T = exp(dmaskT + amask_k), overlaps stages A/B ----
    for kc in range(NKC):
        dm = cstg.tile([128, S], F32, tag="dm")
        nc.sync.dma_start(
            out=dm[:], in_=h["dmaskT"].ap()[kc * 128 : (kc + 1) * 128, :]
        )
        nc.scalar.activation(ET[:, kc, :], dm[:], Exp, bias=amask_sb[:, kc : kc + 1])

    # ---- stages A+B: load (cast-DMA) + projections ----
    with tc.tile_pool(name="stageAB", bufs=1) as sab, \
         tc.tile_pool(name="proj_psum", bufs=2, space="PSUM") as pps:
        xTb = sab.tile([128, NIC, S], BF16)
        wqb = sab.tile([128, NIC, EC], BF16)
        wkb = sab.tile([128, NIC, EC], BF16)
        wvb = sab.tile([128, NIC, EC], BF16)
        for ic in range(NIC):
            nc.gpsimd.dma_start(
                out=xTb[:, ic, :], in_=h["xT"].ap()[ic * 128 : (ic + 1) * 128, :]
            )
        for name, wtb in (("wqT", wqb), ("wkT", wkb), ("wvT", wvb)):
            for ic in range(NIC):
                nc.gpsimd.dma_start(
                    out=wtb[:, ic, :], in_=h[name].ap()[ic * 128 : (ic + 1) * 128, :]
                )

        for dst, wtb, bias in ((qT, wqb, bq_sb), (kT, wkb, bk_sb)):
            for oc in range(NOC):
                for mq in range(NQQ):
                    ps = pps.tile([128, QW], F32, tag="ps")
                    for ic in range(NIC):
                        nc.tensor.matmul(
                            ps[:],
                            wtb[:, ic, oc * 128 : (oc + 1) * 128],
                            xTb[:, ic, mq * QW : (mq + 1) * QW],
                            start=(ic == 0),
                            stop=(ic == NIC - 1),
                        )
                    nc.vector.tensor_scalar_add(
                        dst[:, oc, mq * QW : (mq + 1) * QW], ps[:], bias[:, oc : oc + 1]
                    )
        for mc in range(NKC):
            vps = pps.tile([128, EC], F32, tag="vps")
            for ic in range(NIC):
                nc.tensor.matmul(
                    vps[:],
                    xTb[:, ic, mc * 128 : (mc + 1) * 128],
                    wvb[:, ic, :],
                    start=(ic == 0),
                    stop=False,
                )
            nc.tensor.matmul(vps[:], ones1[:], bv_sb[:], start=False, stop=True)
            nc.vector.tensor_copy(
                vaug[:, mc, :, 0:D], vps[:].rearrange("p (h d) -> p h d", h=HPC)
            )

    # ---- stage D: attention ----
    with tc.tile_pool(name="s_psum", bufs=3, space="PSUM") as sps, \
         tc.tile_pool(name="dwork", bufs=3) as dwork, \
         tc.tile_pool(name="owork", bufs=2) as owork:

        def tail(S_t, kc, j, qs, ctxA, ctxB):
            ex = dwork.tile([128, 2 * QW], BF16, tag="ex")
            nc.scalar.activation(ex[:], S_t[:], Exp)
            pr = dwork.tile([128, 2 * QW], BF16, tag="pr")
            nc.vector.tensor_mul(pr[:, 0:QW], ex[:, 0:QW], ET[:, kc, qs])
            nc.gpsimd.tensor_mul(pr[:, QW : 2 * QW], ex[:, QW : 2 * QW], ET[:, kc, qs])
            nc.tensor.matmul(
                ctxA[:], vaug[:, kc, 2 * j, 0 : D + 1], pr[:, 0:QW],
                start=(kc == 0), stop=(kc == NKC - 1),
            )
            nc.tensor.matmul(
                ctxB[:], vaug[:, kc, 2 * j + 1, 0 : D + 1], pr[:, QW : 2 * QW],
                start=(kc == 0), stop=(kc == NKC - 1),
            )

        for qq in range(NQQ):
            qs = slice(qq * QW, (qq + 1) * QW)
            osb_t = owork.tile([128, 4, EC], F32, tag="osb")
            for j in range(NOC):
                csbs = []
                with tc.tile_pool(name="ctxp", bufs=1, space="PSUM") as cps:
                    ctxA = cps.tile([D + 1, QW], F32, tag="cA")
                    ctxB = cps.tile([D + 1, QW], F32, tag="cB")
                    prev = None
                    for kc in range(NKC):
                        ks = slice(kc * 128, (kc + 1) * 128)
                        S_t = sps.tile([128, 2 * QW], F32, tag="S")
                        nc.tensor.matmul(
                            S_t[:, 0:QW], kT[0:64, j, ks], qT[0:64, j, qs],
                            start=True, stop=True, tile_position=(0, 0),
                        )
                        nc.tensor.matmul(
                            S_t[:, QW : 2 * QW], kT[64:128, j, ks], qT[64:128, j, qs],
                            start=True, stop=True, tile_position=(64, 0),
                        )
                        if prev is not None:
                            tail(prev[0], prev[1], j, qs, ctxA, ctxB)
                        prev = (S_t, kc)
                    tail(prev[0], prev[1], j, qs, ctxA, ctxB)
                    for cpsum in (ctxA, ctxB):
                        csb = dwork.tile([D + 1, QW], BF16, tag="csb")
                        nc.vector.tensor_copy(csb[:], cpsum[:])
                        csbs.append(csb)
                with tc.tile_pool(name="tpp", bufs=2, space="PSUM") as tpp:
                    for hh, csb in enumerate(csbs):
                        tp = tpp.tile([128, 4, D + 4], BF16, tag="tp")
                        for t in range(4):
                            nc.tensor.transpose(
                                tp[:, t, 0 : D + 1],
                                csb[:, t * 128 : (t + 1) * 128],
                                idt[0 : D + 1, 0 : D + 1],
                            )
                        rc4 = dwork.tile([128, 4], F32, tag="rc4")
                        nc.vector.reciprocal(
                            rc4[:], tp[:, :, D : D + 1].rearrange("p a b -> p (a b)")
                        )
                        col = (2 * j + hh) * D
                        nc.vector.tensor_tensor(
                            osb_t[:, :, col : col + D],
                            tp[:, :, 0:D],
                            _bcast_last(rc4[:], D),
                            op=mybir.AluOpType.mult,
                        )
            nc.sync.dma_start(
                out=h["out"].ap()[qq * QW : (qq + 1) * QW, :].rearrange(
                    "(t p) e -> p t e", p=128
                ),
                in_=osb_t[:],
            )


def build():
    nc = bacc.Bacc("TRN2", target_bir_lowering=False, debug=False, num_devices=N_CORES)
    h = {
        "xT": nc.dram_tensor("xT", [E, S], F32, kind="ExternalInput"),
        "wqT": nc.dram_tensor("wqT", [E, EC], F32, kind="ExternalInput"),
        "wkT": nc.dram_tensor("wkT", [E, EC], F32, kind="ExternalInput"),
        "wvT": nc.dram_tensor("wvT", [E, EC], F32, kind="ExternalInput"),
        "bq": nc.dram_tensor("bq", [128, NOC], F32, kind="ExternalInput"),
        "bk": nc.dram_tensor("bk", [128, NOC], F32, kind="ExternalInput"),
        "bv": nc.dram_tensor("bv", [1, EC], F32, kind="ExternalInput"),
        "amask": nc.dram_tensor("amask", [128, NKC], F32, kind="ExternalInput"),
        "dmaskT": nc.dram_tensor("dmaskT", [S, S], F32, kind="ExternalInput"),
        "out": nc.dram_tensor("out", [S, EC], F32, kind="ExternalOutput"),
    }
    with tile.TileContext(nc) as tc:
        with ExitStack() as ctx:
            _emit(ctx, tc, h)
    nc.compile()
    return nc


def prep_in_maps(inputs):
    hs = np.asarray(inputs["hidden_states"], dtype=np.float32)
    am = np.asarray(inputs["attention_mask"], dtype=np.float32)
    dm = np.asarray(inputs["domain_attn_mask"], dtype=np.float32)
    Wq = np.asarray(inputs["Wq"], dtype=np.float32)
    bq = np.asarray(inputs["bq"], dtype=np.float32)
    Wk = np.asarray(inputs["Wk"], dtype=np.float32)
    bk = np.asarray(inputs["bk"], dtype=np.float32)
    Wv = np.asarray(inputs["Wv"], dtype=np.float32)
    bv = np.asarray(inputs["bv"], dtype=np.float32)

    in_maps = []
    for c in range(N_CORES):
        b = c // 2
        e0 = (c % 2) * EC
        sl = slice(e0, e0 + EC)
        in_maps.append(
            {
                "xT": np.ascontiguousarray(hs[b].T),
                "wqT": np.ascontiguousarray(Wq[sl, :].T) * 0.125,
                "wkT": np.ascontiguousarray(Wk[sl, :].T),
                "wvT": np.ascontiguousarray(Wv[sl, :].T),
                "bq": np.ascontiguousarray((bq[sl] * 0.125).reshape(NOC, 128).T),
                "bk": np.ascontiguousarray(bk[sl].reshape(NOC, 128).T),
                "bv": bv[sl].reshape(1, EC).copy(),
                "amask": np.ascontiguousarray(am[b, 0, 0, :].reshape(NKC, 128).T),
                "dmaskT": np.ascontiguousarray(dm[b, 0].T),
            }
        )
    return in_maps


_cached_nc = None


def run(inputs, trace=False):
    global _cached_nc
    if _cached_nc is None:
        _cached_nc = build()
    in_maps = prep_in_maps(inputs)
    res = run_bass_kernel_spmd(
        _cached_nc, in_maps, core_ids=list(range(N_CORES)), trace=trace
    )
    out = np.empty((B, S, E), dtype=np.float32)
    for c in range(N_CORES):
        b = c // 2
        e0 = (c % 2) * EC
        out[b, :, e0 : e0 + EC] = res.results[c]["out"]
    return out, res


def kernel(**inputs) -> np.ndarray:
    return run(inputs)[0]


# revision 6
# speedup vs baseline: 1.3080x; 1.1144x over previous
"""BERT self-attention (B=4, S=2048, E=768, H=12) on 8 TRN2 NeuronCores.

Sharding: (batch, head-half) — core c handles batch c//2, heads 6*(c%2)..+6.
Each core is fully independent (no collectives).

Device-side structure (per core):
  - inputs arrive pre-transposed from host: xT [768,2048], W*T [768,384]
    (Wq/bq pre-scaled by 1/8 = 1/sqrt(D)), dmaskT [2048,2048].
  - x/W loaded via SWDGE cast-DMA (f32 HBM -> bf16 SBUF) on the gpsimd
    queue; dmaskT streamed f32 on the sync queue (separate DMA queues).
  - projections (bf16): qT,kT in [o,m] layout; v in [m,o] layout augmented
    with a ones column per head (softmax denominators via the PV matmul).
  - scores^T[k,q] = kT.T @ qT, two heads row-packed per PE pass (d=64 each).
  - one ACT pass per k-chunk: exp(scores) PSUM -> SBUF bf16 (bottleneck).
  - masks enter via E_T = exp(dmaskT + amask_k) (ACT, overlapped with the
    projections), multiplied in at bf16: head A on DVE, head B on GpSimd.
  - PV: ctx_u^T[65,q] = v_aug.T @ prod accumulated over k-chunks; row 64 is
    the softmax denominator.
  - PE-transpose ctx_u^T -> [q,65] (batched per 4 q-subtiles), divide rows
    0..63 by row 64 (DVE reciprocal + one broadcast multiply), one DMA per
    512-row output stripe.
"""

import sys

if "/opt/trn_rl_repo" not in sys.path:
    sys.path.insert(0, "/opt/trn_rl_repo")

from contextlib import ExitStack

import numpy as np

import concourse.bass as bass
import concourse.tile as tile
from concourse import bacc, mybir
from concourse.bass_utils import run_bass_kernel_spmd
from concourse.masks import make_identity

B, S, E, H = 4, 2048, 768, 12
D = 64
N_CORES = 8
HPC = 6            # heads per core
EC = HPC * D       # 384 embedding cols per core
NIC = E // 128     # 6 contraction chunks
NOC = EC // 128    # 3 output chunks (= head pairs)
NKC = S // 128     # 16 k chunks
QW = 512           # q tile width
NQQ = S // QW      # 4 q chunks

F32 = mybir.dt.float32
BF16 = mybir.dt.bfloat16
Exp = mybir.ActivationFunctionType.Exp


def _bcast_last(ap: bass.AP, n: int) -> bass.AP:
    """Append a step-0 broadcast dim of size n to an AP."""
    return bass.AP(tensor=ap.tensor, offset=ap.offset, ap=[*ap.ap, [0, n]])


def _emit(ctx: ExitStack, tc: tile.TileContext, h):
    nc = tc.nc

    persist = ctx.enter_context(tc.tile_pool(name="persist", bufs=1))
    consts = ctx.enter_context(tc.tile_pool(name="consts", bufs=1))

    # ---- constants ----
    idt = consts.tile([128, 128], BF16)
    make_identity(nc, idt[:])
    amask_sb = consts.tile([128, NKC], F32)
    nc.sync.dma_start(out=amask_sb[:], in_=h["amask"].ap())
    bq_sb = consts.tile([128, NOC], F32)
    nc.sync.dma_start(out=bq_sb[:], in_=h["bq"].ap())
    bk_sb = consts.tile([128, NOC], F32)
    nc.sync.dma_start(out=bk_sb[:], in_=h["bk"].ap())
    bv_sb = consts.tile([1, EC], BF16)
    nc.gpsimd.dma_start(out=bv_sb[:], in_=h["bv"].ap())  # cast f32->bf16
    ones1 = consts.tile([1, 128], BF16)
    nc.vector.memset(ones1[:], 1.0)

    # ---- persistent activations ----
    qT = persist.tile([128, NOC, S], BF16)        # [o%128, o-chunk, m]
    kT = persist.tile([128, NOC, S], BF16)
    vaug = persist.tile([128, NKC, HPC, D + 4], BF16)  # [m%128, m-chunk, head, d|one]
    ET = persist.tile([128, NKC, S], BF16)        # exp(dmaskT + amask), [k%128, k-chunk, q]

    nc.vector.memset(vaug[:, :, :, D : D + 1], 1.0)

    cstg = ctx.enter_context(tc.tile_pool(name="cstg", bufs=2))

    # ---- stage C: E_T = exp(dmaskT + amask_k), overlaps stages A/B ----
    for kc in range(NKC):
        dm = cstg.tile([128, S], F32, tag="dm")
        nc.sync.dma_start(
            out=dm[:], in_=h["dmaskT"].ap()[kc * 128 : (kc + 1) * 128, :]
        )
        nc.scalar.activation(ET[:, kc, :], dm[:], Exp, bias=amask_sb[:, kc : kc + 1])

    # ---- stages A+B: load (cast-DMA) + projections ----
    with tc.tile_pool(name="stageAB", bufs=1) as sab, \
         tc.tile_pool(name="proj_psum", bufs=2, space="PSUM") as pps:
        xTb = sab.tile([128, NIC, S], BF16)
        wqb = sab.tile([128, NIC, EC], BF16)
        wkb = sab.tile([128, NIC, EC], BF16)
        wvb = sab.tile([128, NIC, EC], BF16)
        for ic in range(NIC):
            nc.gpsimd.dma_start(
                out=xTb[:, ic, :], in_=h["xT"].ap()[ic * 128 : (ic + 1) * 128, :]
            )
        for name, wtb in (("wqT", wqb), ("wkT", wkb), ("wvT", wvb)):
            for ic in range(NIC):
                nc.gpsimd.dma_start(
                    out=wtb[:, ic, :], in_=h[name].ap()[ic * 128 : (ic + 1) * 128, :]
                )

        def proj_qk(dst, wtb, bias, oc):
            for mq in range(NQQ):
                ps = pps.tile([128, QW], F32, tag="ps")
                for ic in range(NIC):
                    nc.tensor.matmul(
                        ps[:],
                        wtb[:, ic, oc * 128 : (oc + 1) * 128],
                        xTb[:, ic, mq * QW : (mq + 1) * QW],
                        start=(ic == 0),
                        stop=(ic == NIC - 1),
                    )
                nc.vector.tensor_scalar_add(
                    dst[:, oc, mq * QW : (mq + 1) * QW], ps[:], bias[:, oc : oc + 1]
                )

        def proj_v(mc):
            vps = pps.tile([128, EC], F32, tag="vps")
            for ic in range(NIC):
                nc.tensor.matmul(
                    vps[:],
                    xTb[:, ic, mc * 128 : (mc + 1) * 128],
                    wvb[:, ic, :],
                    start=(ic == 0),
                    stop=False,
                )
            nc.tensor.matmul(vps[:], ones1[:], bv_sb[:], start=False, stop=True)
            nc.vector.tensor_copy(
                vaug[:, mc, :, 0:D], vps[:].rearrange("p (h d) -> p h d", h=HPC)
            )

        # j=0 needs qT/kT chunk 0 + vaug; emit those first so stage D can start
        proj_qk(qT, wqb, bq_sb, 0)
        proj_qk(kT, wkb, bk_sb, 0)
        for mc in range(NKC):
            proj_v(mc)
        for oc in range(1, NOC):
            proj_qk(qT, wqb, bq_sb, oc)
            proj_qk(kT, wkb, bk_sb, oc)

    # ---- stage D: attention ----
    with tc.tile_pool(name="s_psum", bufs=3, space="PSUM") as sps, \
         tc.tile_pool(name="dwork", bufs=3) as dwork, \
         tc.tile_pool(name="owork", bufs=2) as owork:

        def tail(S_t, kc, j, qs, ctxA, ctxB):
            ex = dwork.tile([128, 2 * QW], BF16, tag="ex")
            nc.scalar.activation(ex[:], S_t[:], Exp)
            pr = dwork.tile([128, 2 * QW], BF16, tag="pr")
            nc.vector.tensor_mul(pr[:, 0:QW], ex[:, 0:QW], ET[:, kc, qs])
            nc.vector.tensor_mul(pr[:, QW : 2 * QW], ex[:, QW : 2 * QW], ET[:, kc, qs])
            nc.tensor.matmul(
                ctxA[:], vaug[:, kc, 2 * j, 0 : D + 1], pr[:, 0:QW],
                start=(kc == 0), stop=(kc == NKC - 1),
            )
            nc.tensor.matmul(
                ctxB[:], vaug[:, kc, 2 * j + 1, 0 : D + 1], pr[:, QW : 2 * QW],
                start=(kc == 0), stop=(kc == NKC - 1),
            )

        for qq in range(NQQ):
            qs = slice(qq * QW, (qq + 1) * QW)
            osb_t = owork.tile([128, 4, EC], F32, tag="osb")
            for j in range(NOC):
                csbs = []
                with tc.tile_pool(name="ctxp", bufs=1, space="PSUM") as cps:
                    ctxA = cps.tile([D + 1, QW], F32, tag="cA")
                    ctxB = cps.tile([D + 1, QW], F32, tag="cB")
                    prev = None
                    for kc in range(NKC):
                        ks = slice(kc * 128, (kc + 1) * 128)
                        S_t = sps.tile([128, 2 * QW], F32, tag="S")
                        nc.tensor.matmul(
                            S_t[:, 0:QW], kT[0:64, j, ks], qT[0:64, j, qs],
                            start=True, stop=True, tile_position=(0, 0),
                        )
                        nc.tensor.matmul(
                            S_t[:, QW : 2 * QW], kT[64:128, j, ks], qT[64:128, j, qs],
                            start=True, stop=True, tile_position=(64, 0),
                        )
                        if prev is not None:
                            tail(prev[0], prev[1], j, qs, ctxA, ctxB)
                        prev = (S_t, kc)
                    tail(prev[0], prev[1], j, qs, ctxA, ctxB)
                    for cpsum in (ctxA, ctxB):
                        csb = dwork.tile([D + 1, QW], BF16, tag="csb")
                        nc.vector.tensor_copy(csb[:], cpsum[:])
                        csbs.append(csb)
                with tc.tile_pool(name="tpp", bufs=2, space="PSUM") as tpp:
                    for hh, csb in enumerate(csbs):
                        tp = tpp.tile([128, 4, D + 4], BF16, tag="tp")
                        for t in range(4):
                            nc.tensor.transpose(
                                tp[:, t, 0 : D + 1],
                                csb[:, t * 128 : (t + 1) * 128],
                                idt[0 : D + 1, 0 : D + 1],
                            )
                        rc4 = dwork.tile([128, 4], F32, tag="rc4")
                        nc.vector.reciprocal(
                            rc4[:], tp[:, :, D : D + 1].rearrange("p a b -> p (a b)")
                        )
                        col = (2 * j + hh) * D
                        nc.vector.tensor_tensor(
                            osb_t[:, :, col : col + D],
                            tp[:, :, 0:D],
                            _bcast_last(rc4[:], D),
                            op=mybir.AluOpType.mult,
                        )
            nc.sync.dma_start(
                out=h["out"].ap()[qq * QW : (qq + 1) * QW, :].rearrange(
                    "(t p) e -> p t e", p=128
                ),
                in_=osb_t[:],
            )


def build():
    nc = bacc.Bacc("TRN2", target_bir_lowering=False, debug=False, num_devices=N_CORES)
    h = {
        "xT": nc.dram_tensor("xT", [E, S], F32, kind="ExternalInput"),
        "wqT": nc.dram_tensor("wqT", [E, EC], F32, kind="ExternalInput"),
        "wkT": nc.dram_tensor("wkT", [E, EC], F32, kind="ExternalInput"),
        "wvT": nc.dram_tensor("wvT", [E, EC], F32, kind="ExternalInput"),
        "bq": nc.dram_tensor("bq", [128, NOC], F32, kind="ExternalInput"),
        "bk": nc.dram_tensor("bk", [128, NOC], F32, kind="ExternalInput"),
        "bv": nc.dram_tensor("bv", [1, EC], F32, kind="ExternalInput"),
        "amask": nc.dram_tensor("amask", [128, NKC], F32, kind="ExternalInput"),
        "dmaskT": nc.dram_tensor("dmaskT", [S, S], F32, kind="ExternalInput"),
        "out": nc.dram_tensor("out", [S, EC], F32, kind="ExternalOutput"),
    }
    with tile.TileContext(nc) as tc:
        with ExitStack() as ctx:
            _emit(ctx, tc, h)
    nc.compile()
    return nc


def prep_in_maps(inputs):
    hs = np.asarray(inputs["hidden_states"], dtype=np.float32)
    am = np.asarray(inputs["attention_mask"], dtype=np.float32)
    dm = np.asarray(inputs["domain_attn_mask"], dtype=np.float32)
    Wq = np.asarray(inputs["Wq"], dtype=np.float32)
    bq = np.asarray(inputs["bq"], dtype=np.float32)
    Wk = np.asarray(inputs["Wk"], dtype=np.float32)
    bk = np.asarray(inputs["bk"], dtype=np.float32)
    Wv = np.asarray(inputs["Wv"], dtype=np.float32)
    bv = np.asarray(inputs["bv"], dtype=np.float32)

    in_maps = []
    for c in range(N_CORES):
        b = c // 2
        e0 = (c % 2) * EC
        sl = slice(e0, e0 + EC)
        in_maps.append(
            {
                "xT": np.ascontiguousarray(hs[b].T),
                "wqT": np.ascontiguousarray(Wq[sl, :].T) * 0.125,
                "wkT": np.ascontiguousarray(Wk[sl, :].T),
                "wvT": np.ascontiguousarray(Wv[sl, :].T),
                "bq": np.ascontiguousarray((bq[sl] * 0.125).reshape(NOC, 128).T),
                "bk": np.ascontiguousarray(bk[sl].reshape(NOC, 128).T),
                "bv": bv[sl].reshape(1, EC).copy(),
                "amask": np.ascontiguousarray(am[b, 0, 0, :].reshape(NKC, 128).T),
                "dmaskT": np.ascontiguousarray(dm[b, 0].T),
            }
        )
    return in_maps


_cached_nc = None


def run(inputs, trace=False):
    global _cached_nc
    if _cached_nc is None:
        _cached_nc = build()
    in_maps = prep_in_maps(inputs)
    res = run_bass_kernel_spmd(
        _cached_nc, in_maps, core_ids=list(range(N_CORES)), trace=trace
    )
    out = np.empty((B, S, E), dtype=np.float32)
    for c in range(N_CORES):
        b = c // 2
        e0 = (c % 2) * EC
        out[b, :, e0 : e0 + EC] = res.results[c]["out"]
    return out, res


def kernel(**inputs) -> np.ndarray:
    return run(inputs)[0]
